# revision 12
# baseline (speedup 1.0000x reference)
"""Trainium2 Bass kernel for nn_AttentionHead (pre-softmax scores variant).

The module returns (q @ k^T * scale) @ v with NO softmax, so the product is
associative:  out = (scale*q) @ (k^T @ v)  with k^T @ v a tiny [64, 64]
matrix.  This removes the [T, T] score matrix entirely: the kernel streams
x once, computes q/k/v projections, a [64, 64] partial Gram of (k, v), a
pairwise AllReduce across the two cores holding each batch, and one final
tall-skinny matmul.

Sharding: core c <- (batch b = c//2, sequence half h = c%2), 2048 tokens per
core.  Partial S = k^T v is AllReduce-summed within core pairs
[[0,1],[2,3],[4,5],[6,7]].

Host-side marshalling transposes each core's x-chunk so the kernel reads
x^T tiles (contraction dim on partitions) straight from DRAM, and folds the
softmax scale into Wq/bq.
"""

import sys

sys.path.insert(0, "/opt/trn_rl_repo")

import numpy as np

B, T, C, H = 4, 4096, 768, 64
N_CORES = 8
TPC = T // 2  # tokens per core (half a batch's sequence)
CI = C // 128  # 6 contraction chunks
NT = TPC // 512  # 4 moving-dim slices for projections
TI = TPC // 128  # 16 token tiles
SCALE = float(C) ** -0.5

# float32r streams fp32 matmuls at full rate but the PE reduces operand
# precision (~1e-3 relative on hardware); plain float32 runs at 1/4 rate
# but is exact.  Default mode: 3-pass bf16 split-GEMM — x and W are split
# host-side into bf16 hi+lo pairs and the projection runs as
# x_hi@W_hi + x_hi@W_lo + x_lo@W_hi with fp32 PSUM accumulation (exact to
# ~5e-6 relative, 3 cycles/row instead of fp32's 4, same DMA bytes).
USE_F32R = False
USE_BF16_SPLIT = True

_CACHE = {}


def _patch_tile_drain():
    """This walrus build rejects >1 sync wait on TPB_CTRL instructions
    (Drain/NoOp) and the butterfly barrier rides eq-waits on drains.
    Replace the TileContext exit sequence with single-wait nops + plain
    drain + sem-only barriers."""
    import bass_rust as _bass_rust
    import concourse.tile as tile
    from concourse.vector_clock import ScopedClock

    def _drain_and_barrier(self, tick_clock, wait_clock):
        nc = self.nc
        probe = nc.sync.nop(nofuse=True)
        wait_clock.add_sem_waits(
            probe.ins, ScopedClock({None: tick_clock.global_clock})
        )
        waits = list(probe.ins.sync_info.on_wait) if probe.ins.sync_info else []
        updates = list(probe.ins.sync_info.on_update) if probe.ins.sync_info else []
        probe.ins.sync_info = _bass_rust.SyncInfo(
            on_wait=waits[:1], on_update=updates
        )
        for i in range(1, len(waits)):
            extra = nc.sync.nop(nofuse=True)
            extra.ins.sync_info = _bass_rust.SyncInfo(
                on_wait=waits[i : i + 1], on_update=[]
            )
        nc.sync.drain()
        nc.all_engine_barrier(sem_only=True)
        popped = nc._tile_sem_poison_stack.pop()
        assert popped is self._sem_poison
        nc.clear_and_free_semaphores(list(self.sems.allocated().values()))
        nc.all_engine_barrier(sem_only=True)

    tile.TileContext._drain_and_barrier = _drain_and_barrier


def _split_multi_waits(nc):
    """This walrus build allows only ONE sync-wait command per regular
    instruction.  Move extra waits onto dedicated same-engine NOPs placed
    immediately before the instruction (an engine blocks on its own stream,
    so this is semantically identical)."""
    import bass_rust
    import concourse.mybir as mybir

    cnt = 0
    for fn in nc.m.functions:
        for bb in fn.blocks:
            out = []
            for ins in bb.instructions:
                si = ins.sync_info
                if si is not None and si.on_wait and len(si.on_wait) > 1:
                    waits = list(si.on_wait)
                    for w in waits[:-1]:
                        nop = mybir.InstNoOp(name=f"I-waitsplit-{cnt}")
                        cnt += 1
                        nop.engine = ins.engine
                        nop.bass_nofuse = True
                        nop.sync_info = bass_rust.SyncInfo(
                            on_wait=[w], on_update=[]
                        )
                        out.append(nop)
                    ins.sync_info = bass_rust.SyncInfo(
                        on_wait=[waits[-1]], on_update=list(si.on_update or [])
                    )
                out.append(ins)
            bb.instructions = out
    return cnt


def _build_nc(no_collective=False):
    import concourse.bass as bass
    import concourse.mybir as mybir
    import concourse.tile as tile
    from bass_rust import add_dep_helper

    _patch_tile_drain()

    f32 = mybir.dt.float32
    bf16 = mybir.dt.bfloat16
    fact = mybir.dt.float32r if USE_F32R else f32

    nc = bass.Bass("TRN2", target_bir_lowering=False, debug=False, num_devices=N_CORES)

    if USE_BF16_SPLIT:
        xh = nc.dram_tensor("xh", [128, CI, TPC], bf16, kind="ExternalInput").ap()
        xl = nc.dram_tensor("xl", [128, CI, TPC], bf16, kind="ExternalInput").ap()
        wqkh = nc.dram_tensor("wqkh", [128, CI, 128], bf16, kind="ExternalInput").ap()
        wqkl = nc.dram_tensor("wqkl", [128, CI, 128], bf16, kind="ExternalInput").ap()
        wvh = nc.dram_tensor("wvh", [128, CI, H], bf16, kind="ExternalInput").ap()
        wvl = nc.dram_tensor("wvl", [128, CI, H], bf16, kind="ExternalInput").ap()
    else:
        xt = nc.dram_tensor("xt", [128, CI, TPC], fact, kind="ExternalInput").ap()
        wqk = nc.dram_tensor("wqk", [128, CI, 128], fact, kind="ExternalInput").ap()
        wv = nc.dram_tensor("wv", [128, CI, H], fact, kind="ExternalInput").ap()
    bqk = nc.dram_tensor("bqk", [128, 1], f32, kind="ExternalInput").ap()
    bvp = nc.dram_tensor("bv", [H, 1], f32, kind="ExternalInput").ap()
    ident = nc.dram_tensor("ident", [128, 128], f32, kind="ExternalInput").ap()
    out = nc.dram_tensor("out", [128, TI, H], f32, kind="ExternalOutput").ap()
    cc_in = nc.dram_tensor("cc_in", [H, H], f32)
    cc_out = nc.dram_tensor("cc_out", [2, H, H], f32)
    RG = [[0, 1], [2, 3], [4, 5], [6, 7]]

    with tile.TileContext(nc) as tc:
        with (
            tc.tile_pool(name="const", bufs=1) as cpool,
            tc.tile_pool(name="data", bufs=1) as dpool,
            tc.tile_pool(name="work", bufs=2) as wpool,
            tc.tile_pool(name="psum", bufs=4, space="PSUM") as ppool,
        ):
            bqk_sb = cpool.tile([128, 1], f32)
            nc.sync.dma_start(out=bqk_sb[:], in_=bqk)
            bv_sb = cpool.tile([H, 1], f32)
            nc.sync.dma_start(out=bv_sb[:], in_=bvp)
            id_sb = cpool.tile([128, 128], f32)
            nc.sync.dma_start(out=id_sb[:], in_=ident)

            if USE_BF16_SPLIT:
                wqkh_sb = cpool.tile([128, CI, 128], bf16)
                nc.sync.dma_start(out=wqkh_sb[:], in_=wqkh)
                wqkl_sb = cpool.tile([128, CI, 128], bf16)
                nc.sync.dma_start(out=wqkl_sb[:], in_=wqkl)
                wvh_sb = cpool.tile([128, CI, H], bf16)
                nc.sync.dma_start(out=wvh_sb[:], in_=wvh)
                wvl_sb = cpool.tile([128, CI, H], bf16)
                nc.sync.dma_start(out=wvl_sb[:], in_=wvl)
                xh_sb = dpool.tile([128, CI, TPC], bf16)
                xl_sb = dpool.tile([128, CI, TPC], bf16)
                for ci in range(CI):
                    nc.sync.dma_start(out=xh_sb[:, ci, :], in_=xh[:, ci, :])
                    nc.sync.dma_start(out=xl_sb[:, ci, :], in_=xl[:, ci, :])
            else:
                wqk_sb = cpool.tile([128, CI, 128], fact)
                nc.sync.dma_start(out=wqk_sb[:], in_=wqk)
                wv_sb = cpool.tile([128, CI, H], fact)
                nc.sync.dma_start(out=wv_sb[:], in_=wv)
                xt_sb = dpool.tile([128, CI, TPC], fact)
                for ci in range(CI):
                    nc.sync.dma_start(out=xt_sb[:, ci, :], in_=xt[:, ci, :])

            # Projections: qk^T = (Wq*scale | Wk)^T x^T + bias, v^T likewise.
            # qkT rows 0..63 hold scale*q^T, rows 64..127 hold k^T.
            qkT = dpool.tile([128, TPC], f32)
            vT = dpool.tile([H, TPC], f32)
            psum_qk = [
                ppool.tile([128, 512], f32, tag="A", name=f"pqk{nt}")
                for nt in range(NT)
            ]
            psum_v = [
                ppool.tile([H, 512], f32, tag="B", name=f"pv{nt}") for nt in range(NT)
            ]
            if USE_BF16_SPLIT:
                for ci in range(CI):
                    for nt in range(NT):
                        sl = slice(nt * 512, (nt + 1) * 512)
                        first = ci == 0
                        last = ci == CI - 1
                        nc.tensor.matmul(
                            psum_qk[nt][:], wqkh_sb[:, ci, :], xh_sb[:, ci, sl],
                            start=first, stop=False,
                        )
                        nc.tensor.matmul(
                            psum_qk[nt][:], wqkl_sb[:, ci, :], xh_sb[:, ci, sl],
                            start=False, stop=False,
                        )
                        nc.tensor.matmul(
                            psum_qk[nt][:], wqkh_sb[:, ci, :], xl_sb[:, ci, sl],
                            start=False, stop=last,
                        )
                    for nt in range(NT):
                        sl = slice(nt * 512, (nt + 1) * 512)
                        first = ci == 0
                        last = ci == CI - 1
                        nc.tensor.matmul(
                            psum_v[nt][:], wvh_sb[:, ci, :], xh_sb[:, ci, sl],
                            start=first, stop=False,
                        )
                        nc.tensor.matmul(
                            psum_v[nt][:], wvl_sb[:, ci, :], xh_sb[:, ci, sl],
                            start=False, stop=False,
                        )
                        nc.tensor.matmul(
                            psum_v[nt][:], wvh_sb[:, ci, :], xl_sb[:, ci, sl],
                            start=False, stop=last,
                        )
            else:
                for ci in range(CI):
                    for nt in range(NT):
                        nc.tensor.matmul(
                            psum_qk[nt][:],
                            wqk_sb[:, ci, :],
                            xt_sb[:, ci, nt * 512 : (nt + 1) * 512],
                            start=(ci == 0),
                            stop=(ci == CI - 1),
                        )
                    for nt in range(NT):
                        nc.tensor.matmul(
                            psum_v[nt][:],
                            wv_sb[:, ci, :],
                            xt_sb[:, ci, nt * 512 : (nt + 1) * 512],
                            start=(ci == 0),
                            stop=(ci == CI - 1),
                        )
            for nt in range(NT):
                sl = slice(nt * 512, (nt + 1) * 512)
                nc.vector.tensor_add(
                    out=qkT[:, sl],
                    in0=psum_qk[nt][:],
                    in1=bqk_sb.to_broadcast((128, 512)),
                )
                nc.vector.tensor_add(
                    out=vT[:, sl],
                    in0=psum_v[nt][:],
                    in1=bv_sb.to_broadcast((H, 512)),
                )

            # Back-transpose k^T, v^T to token-major for the S contraction.
            k_nat = dpool.tile([128, TI, H], f32)
            v_nat = dpool.tile([128, TI, H], f32)
            for ti in range(TI):
                tsl = slice(ti * 128, (ti + 1) * 128)
                pqk_t = ppool.tile([128, 128], f32, tag="A", name="pqkt")
                nc.tensor.transpose(pqk_t[:], qkT[:, tsl], id_sb[:])
                nc.vector.tensor_copy(out=k_nat[:, ti, :], in_=pqk_t[:, 64:128])
                pv_t = ppool.tile([128, H], f32, tag="B", name="pvt")
                nc.tensor.transpose(pv_t[:], vT[:, tsl], id_sb[0:H, 0:H])
                nc.vector.tensor_copy(out=v_nat[:, ti, :], in_=pv_t[:])

            # Partial S = k^T v over this core's 2048 tokens.
            psum_s = ppool.tile([H, H], f32, tag="B", name="ps")
            for ti in range(TI):
                nc.tensor.matmul(
                    psum_s[:],
                    k_nat[:, ti, :],
                    v_nat[:, ti, :],
                    start=(ti == 0),
                    stop=(ti == TI - 1),
                )
            s_sb = wpool.tile([H, H], f32, tag="s")
            nc.vector.tensor_copy(out=s_sb[:], in_=psum_s[:])
            dma_to_cc = nc.sync.dma_start(out=cc_in.ap(), in_=s_sb[:])
            sf_sb = wpool.tile([H, H], f32, tag="sf")
            if no_collective:
                dma_from_cc = nc.sync.dma_start(out=sf_sb[:], in_=cc_in.ap())
                add_dep_helper(
                    dma_from_cc.ins, dma_to_cc.ins, reason="S readback after write"
                )
            else:
                # AllGather (lower latency floor than AllReduce) + on-chip
                # add of the two partial S matrices.
                cc = nc.gpsimd.collective_compute(
                    "AllGather",
                    mybir.AluOpType.bypass,
                    replica_groups=RG,
                    ins=[cc_in.ap()],
                    outs=[cc_out.ap()],
                )
                add_dep_helper(
                    cc.ins, dma_to_cc.ins, reason="collective waits for S DMA"
                )
                sg_sb = wpool.tile([H, 2, H], f32, tag="sg")
                dma_from_cc = nc.sync.dma_start(
                    out=sg_sb[:], in_=cc_out.ap().rearrange("r p h -> p r h")
                )
                add_dep_helper(
                    dma_from_cc.ins, cc.ins, reason="S readback waits for collective"
                )
                nc.vector.tensor_add(
                    out=sf_sb[:], in0=sg_sb[:, 0, :], in1=sg_sb[:, 1, :]
                )

            # out = (scale*q) @ S_full, token-major output.
            out_sb = dpool.tile([128, TI, H], f32)
            for ti in range(TI):
                tsl = slice(ti * 128, (ti + 1) * 128)
                po = ppool.tile([128, H], f32, tag="A", name="po")
                nc.tensor.matmul(
                    po[:], qkT[0:H, tsl], sf_sb[:], start=True, stop=True
                )
                nc.vector.tensor_copy(out=out_sb[:, ti, :], in_=po[:])
            nc.sync.dma_start(out=out, in_=out_sb[:])

    _split_multi_waits(nc)
    return nc


def _make_runner():
    """Build the Bass module once and wrap it in a cached, jitted PJRT
    executable (mirrors bass2jax.run_bass_via_pjrt's multi-core path, but
    reusable across calls so repeat invocations skip trace+compile)."""
    import jax
    from jax.experimental.shard_map import shard_map
    from jax.sharding import Mesh, PartitionSpec

    import concourse.mybir as mybir
    from concourse import bass2jax

    nc = _build_nc()
    bass2jax.install_neuronx_cc_hook()

    partition_name = nc.partition_id_tensor.name if nc.partition_id_tensor else None
    in_names, out_names, out_avals, zero_shapes = [], [], [], []
    for alloc in nc.m.functions[0].allocations:
        if not isinstance(alloc, mybir.MemoryLocationSet):
            continue
        name = alloc.memorylocations[0].name
        if alloc.kind == "ExternalInput":
            if name != partition_name:
                in_names.append(name)
        elif alloc.kind == "ExternalOutput":
            out_names.append(name)
            shape = tuple(alloc.tensor_shape)
            dtype = mybir.dt.np(alloc.dtype)
            out_avals.append(jax.core.ShapedArray(shape, dtype))
            zero_shapes.append((shape, dtype))
    n_params = len(in_names)
    in_names_all = list(in_names) + list(out_names)
    if partition_name:
        in_names_all.append(partition_name)

    def _body(*args):
        operands = list(args)
        if partition_name:
            operands.append(bass2jax.partition_id_tensor())
        outs = bass2jax._bass_exec_p.bind(
            *operands,
            out_avals=tuple(out_avals),
            in_names=tuple(in_names_all),
            out_names=tuple(out_names),
            lowering_input_output_aliases=(),
            sim_require_finite=True,
            sim_require_nnan=True,
            nc=nc,
        )
        return tuple(outs)

    devices = jax.devices()[:N_CORES]
    assert len(devices) == N_CORES
    mesh = Mesh(np.asarray(devices), ("core",))
    n_outs = len(out_names)
    sharded = jax.jit(
        shard_map(
            _body,
            mesh=mesh,
            in_specs=(PartitionSpec("core"),) * (n_params + n_outs),
            out_specs=(PartitionSpec("core"),) * n_outs,
            check_rep=False,
        ),
        donate_argnums=tuple(range(n_params, n_params + n_outs)),
        keep_unused=True,
    )
    return {
        "nc": nc,
        "sharded": sharded,
        "in_names": in_names,
        "out_names": out_names,
        "out_avals": out_avals,
        "zero_shapes": zero_shapes,
    }


def _get_runner():
    if "runner" not in _CACHE:
        _CACHE["runner"] = _make_runner()
    return _CACHE["runner"]


def _run(runner, in_maps):
    concat_in = [
        np.concatenate([np.asarray(in_maps[c][nm]) for c in range(N_CORES)], axis=0)
        for nm in runner["in_names"]
    ]
    concat_zeros = [
        np.zeros((N_CORES * shape[0], *shape[1:]), dtype)
        for shape, dtype in runner["zero_shapes"]
    ]
    out_arrs = runner["sharded"](*concat_in, *concat_zeros)
    out_avals = runner["out_avals"]
    return [
        {
            nm: np.asarray(out_arrs[i]).reshape(N_CORES, *out_avals[i].shape)[c]
            for i, nm in enumerate(runner["out_names"])
        }
        for c in range(N_CORES)
    ]


def _bf16_split(a):
    import ml_dtypes

    hi = a.astype(ml_dtypes.bfloat16)
    lo = (a - hi.astype(np.float32)).astype(ml_dtypes.bfloat16)
    return hi, lo


def _prep_inputs(x, Wq, bq, Wk, bk, Wv, bv):
    """Build the 8 per-core input maps (host-side sharding/marshalling)."""
    x = np.asarray(x, dtype=np.float32)
    Wq = np.asarray(Wq, dtype=np.float32)
    Wk = np.asarray(Wk, dtype=np.float32)
    Wv = np.asarray(Wv, dtype=np.float32)
    bq = np.asarray(bq, dtype=np.float32)
    bk = np.asarray(bk, dtype=np.float32)
    bv = np.asarray(bv, dtype=np.float32)

    wqk = np.concatenate([Wq * SCALE, Wk], axis=1)  # [768, 128]
    wqk = np.ascontiguousarray(wqk.reshape(CI, 128, 128).transpose(1, 0, 2))
    wv_r = np.ascontiguousarray(Wv.reshape(CI, 128, H).transpose(1, 0, 2))
    bqk = np.concatenate([bq * SCALE, bk])[:, None].astype(np.float32)
    bv_r = bv[:, None].astype(np.float32)
    ident = np.eye(128, dtype=np.float32)

    common = {"bqk": bqk, "bv": bv_r, "ident": ident}
    if USE_BF16_SPLIT:
        wqkh, wqkl = _bf16_split(wqk)
        wvh, wvl = _bf16_split(wv_r)
        common.update(
            {"wqkh": wqkh, "wqkl": wqkl, "wvh": wvh, "wvl": wvl}
        )
    else:
        common.update({"wqk": wqk, "wv": wv_r})

    in_maps = []
    for c in range(N_CORES):
        b, h = divmod(c, 2)
        xc = x[b, h * TPC : (h + 1) * TPC, :]  # [2048, 768]
        xtc = np.ascontiguousarray(
            xc.T.reshape(CI, 128, TPC).transpose(1, 0, 2)
        )  # [128, CI, 2048]
        m = dict(common)
        if USE_BF16_SPLIT:
            m["xh"], m["xl"] = _bf16_split(xtc)
        else:
            m["xt"] = xtc
        in_maps.append(m)
    return in_maps


def _assemble(results):
    out = np.empty((B, T, H), dtype=np.float32)
    for c in range(N_CORES):
        b, h = divmod(c, 2)
        oc = results[c]["out"]  # [128, TI, 64] partition-major
        out[b, h * TPC : (h + 1) * TPC, :] = oc.transpose(1, 0, 2).reshape(TPC, H)
    return out


def kernel(**inputs):
    runner = _get_runner()
    in_maps = _prep_inputs(**inputs)
    return _assemble(_run(runner, in_maps))
